# revision 8
# baseline (speedup 1.0000x reference)
"""Multi-head attention block (QKV proj + RMSNorm + RoPE + SDPA + out proj)
sharded across 8 Trainium2 NeuronCores — v3.

Sharding: data-parallel over batch (B=2 -> 2 groups of 4 cores), tensor-parallel
over heads (16 heads -> 4 heads/core).  Each core computes a partial output
projection for its 4 heads; the host sums the 4 partials per batch and adds
bproj.

Key measured platform fact driving the design: on this part EVERY ScalarE
instruction costs ~4.3-5us fixed (function/size/source independent), and
PSUM-source DVE ops ~4.2us; so ScalarE work is batched into very few, very
large instructions (8-16 exps of [128, 8192] per core instead of 128 of
[128,1024]) with DVE staging copies PSUM->SBUF, and the attention inner loop
is software-pipelined (attn@v for batch b issues while batch b+1's scores
run).

Changes vs v1:
  - 8-chunk (1024-row) contraction; qkv bias fused into the PSUM->SBUF
    evacuation (per-partition bias for qT/kT, broadcast-tile add for v).
  - RMSNorm stats batched: one [97,512] mask-matmul accumulation per seg per
    head-group ({m0,m2} then {m1,m3}), one Ln per seg, ONE Exp per group for
    all 4 rsqrt rows; cq/ck broadcast via a single [97,128] selection matmul.
  - Denominator broadcast via K=1 ones-row matmul (no zeroed helper rows).
  - Attention inner loop software-pipelined: attn@v for chunk j issues after
    exp(j) while scores(j+1) proceed; exp is the ACT-bound steady state.
  - Reciprocal reads the PSUM denominator row directly; the reciprocal
    broadcast psum reuses the scores-tile ring (no extra PSUM banks).
  - Input DMAs chunked; out-proj copies alternate DVE/ACT and DMA per chunk.
"""

import contextlib

import numpy as np
import ml_dtypes

B, S, D, H = 2, 2048, 1024, 16
HD = D // H
N_CORES = 8
HPC = H // 4  # heads per core = 4
CW = HPC * HD  # per-core head-col width = 256

BF16 = ml_dtypes.bfloat16

LAST_RESULTS = None


def _build_bass(reps=1, debug=False, ablate=()):
    ablate = set(ablate)
    import concourse.mybir as mybir
    import concourse.tile as tile
    from concourse import bacc

    fp32 = mybir.dt.float32
    bf16 = mybir.dt.bfloat16
    AF = mybir.ActivationFunctionType

    nc = bacc.Bacc()

    # ---- DRAM I/O ----
    xT = nc.dram_tensor("xT", [D, S], bf16, kind="ExternalInput")
    wqk = nc.dram_tensor("wqk", [D, 2 * CW], bf16, kind="ExternalInput")
    wv = nc.dram_tensor("wv", [D, CW], bf16, kind="ExternalInput")
    wpr = nc.dram_tensor("wpr", [CW, D], bf16, kind="ExternalInput")
    cosT2 = nc.dram_tensor("cosT2", [128, S], bf16, kind="ExternalInput")
    sinT2 = nc.dram_tensor("sinT2", [128, S], bf16, kind="ExternalInput")
    perm = nc.dram_tensor("perm", [128, 128], bf16, kind="ExternalInput")
    mask97 = nc.dram_tensor("mask97", [128, 2 * 97], bf16, kind="ExternalInput")
    sel97 = nc.dram_tensor("sel97", [97, 2 * 128], bf16, kind="ExternalInput")
    ones1 = nc.dram_tensor("ones1", [1, 64], fp32, kind="ExternalInput")
    bqk = nc.dram_tensor("bqk", [128, 4], fp32, kind="ExternalInput")
    bvb = nc.dram_tensor("bvb", [128, CW], bf16, kind="ExternalInput")
    out = nc.dram_tensor("out", [S, D], fp32, kind="ExternalOutput")
    if debug:
        dbg_qkT = nc.dram_tensor("dbg_qkT", [128, 4 * S], bf16, kind="ExternalOutput")
        dbg_cq = nc.dram_tensor("dbg_cq", [97, 2 * S], bf16, kind="ExternalOutput")
        dbg_v = nc.dram_tensor("dbg_v", [128, 16 * 4 * 66], bf16, kind="ExternalOutput")
        dbg_oT = nc.dram_tensor("dbg_oT", [128, 2 * S], bf16, kind="ExternalOutput")
        dbg_ln = nc.dram_tensor("dbg_ln", [97, 2 * S], fp32, kind="ExternalOutput")

    with tile.TileContext(nc) as tc:
        with tc.tile_pool(name="persist", bufs=1) as pp:
            xT_sb = pp.tile([128, 8, S], bf16, name="xT_sb")
            wqk_sb = pp.tile([128, 8, 2 * CW], bf16, name="wqk_sb")
            wv_sb = pp.tile([128, 8, CW], bf16, name="wv_sb")
            wpr_sb = pp.tile([128, 2, D], bf16, name="wpr_sb")
            cos_sb = pp.tile([128, S], bf16, name="cos_sb")
            sin_sb = pp.tile([128, S], bf16, name="sin_sb")
            perm_sb = pp.tile([128, 128], bf16, name="perm_sb")
            mask_sb = pp.tile([128, 2, 97], bf16, name="mask_sb")
            sel_sb = pp.tile([97, 2, 128], bf16, name="sel_sb")
            ones1_sb = pp.tile([1, 64], fp32, name="ones1_sb")
            bqk_sb = pp.tile([128, 4], fp32, name="bqk_sb")
            bvb_sb = pp.tile([128, CW], bf16, name="bvb_sb")
            qkT_sb = pp.tile([128, 4, S], bf16, name="qkT_sb")
            v_sb = pp.tile([128, 16, 4, 66], bf16, name="v_sb")
            oT_sb = pp.tile([128, 2, S], bf16, name="oT_sb")
            ln_sb = pp.tile([97, 2, S], fp32, name="ln_sb")
            cq_sb = pp.tile([97, 2, S], bf16, name="cq_sb")
            rd_sb = pp.tile([1, 2, 1024], fp32, name="rd_sb")

            rep_stack = contextlib.ExitStack()
            if reps > 1:
                rep_stack.enter_context(tc.For_i(0, reps))

            # ---- input DMAs (chunked so deps are fine-grained) ----
            if "noindma" not in ablate:
                nc.sync.dma_start(wv_sb[:], wv.rearrange("(c p) m -> p c m", p=128))
                for kk in range(8):
                    nc.sync.dma_start(xT_sb[:, kk, :], xT[kk * 128 : (kk + 1) * 128, :])
                nc.sync.dma_start(wqk_sb[:], wqk.rearrange("(c p) m -> p c m", p=128))
            else:
                nc.sync.dma_start(wv_sb[:, :, 0:1], wv.rearrange("(c p) m -> p c m", p=128)[:, :, 0:1])
                for kk in range(8):
                    nc.sync.dma_start(xT_sb[:, kk, 0:1], xT[kk * 128 : (kk + 1) * 128, 0:1])
                nc.sync.dma_start(wqk_sb[:, :, 0:1], wqk.rearrange("(c p) m -> p c m", p=128)[:, :, 0:1])
            nc.sync.dma_start(cos_sb[:], cosT2[:])
            nc.sync.dma_start(sin_sb[:], sinT2[:])
            nc.sync.dma_start(perm_sb[:], perm[:])
            nc.sync.dma_start(mask_sb[:], mask97.rearrange("p (i m) -> p i m", i=2))
            nc.sync.dma_start(sel_sb[:], sel97.rearrange("p (i m) -> p i m", i=2))
            nc.sync.dma_start(ones1_sb[:], ones1[:])
            nc.sync.dma_start(bqk_sb[:], bqk[:])
            nc.sync.dma_start(bvb_sb[:], bvb[:])
            nc.sync.dma_start(wpr_sb[:], wpr.rearrange("(c p) m -> p c m", p=128))
            # ones column for the softmax denominator rider (col 64 of each head)
            nc.vector.memset(v_sb[:, :, :, 64:65], 1.0)

            # ---------- Phase B: V + QKV projections + sum-of-squares ----------
            with (
                tc.tile_pool(name="mmps", bufs=3, space="PSUM") as mmps,
                tc.tile_pool(name="ssps", bufs=1, space="PSUM") as ssps,
                tc.tile_pool(name="sqpool", bufs=3) as sqpool,
            ):
                # v natural: psum[s-chunk] = sum_kk xT[kk,schunk].T @ wv[kk,:]
                for si in range(16):
                    ps = mmps.tile([128, 512], fp32, tag="mm")
                    for kk in range(8):
                        nc.tensor.matmul(
                            ps[:, 0:CW],
                            xT_sb[:, kk, si * 128 : (si + 1) * 128],
                            wv_sb[:, kk, :],
                            start=(kk == 0),
                            stop=(kk == 7),
                        )
                    nc.vector.tensor_add(
                        out=v_sb[:, si, :, 0:64],
                        in0=ps[:, 0:CW].rearrange("p (h c) -> p h c", h=4),
                        in1=bvb_sb[:].rearrange("p (h c) -> p h c", h=4),
                    )

                # q,k transposed with per-group sum-of-squares accumulation.
                # groups: A = (m0 q-heads 0,1 ; m2 k-heads 0,1), B = (m1 ; m3)
                for g in range(2):
                    ss = ssps.tile([97, 4, 512], fp32, tag="ss", name="ss")
                    for mi, m in enumerate((g, 2 + g)):
                        for seg in range(4):
                            ps = mmps.tile([128, 512], fp32, tag="mm")
                            for kk in range(8):
                                nc.tensor.matmul(
                                    ps[:],
                                    wqk_sb[:, kk, m * 128 : (m + 1) * 128],
                                    xT_sb[:, kk, seg * 512 : (seg + 1) * 512],
                                    start=(kk == 0),
                                    stop=(kk == 7),
                                )
                            nc.vector.tensor_scalar_add(
                                out=qkT_sb[:, m, seg * 512 : (seg + 1) * 512],
                                in0=ps[:],
                                scalar1=bqk_sb[:, m : m + 1],
                            )
                            sq = sqpool.tile([128, 512], bf16, tag="sq")
                            qs = qkT_sb[:, m, seg * 512 : (seg + 1) * 512]
                            nc.vector.tensor_mul(out=sq[:], in0=qs, in1=qs)
                            nc.tensor.matmul(
                                ss[:, seg, :],
                                mask_sb[:, mi, :],
                                sq[:],
                                start=(mi == 0),
                                stop=(mi == 1),
                            )
                    # one Ln + one Exp per group (ScalarE ops cost ~4.5us
                    # fixed each on this part, so batch hard)
                    nc.scalar.activation(
                        ln_sb[:, g, :],
                        ss[:].rearrange("p a b -> p (a b)"),
                        AF.Ln,
                        scale=1.0 / HD,
                    )
                    nc.scalar.activation(cq_sb[:, g, :], ln_sb[:, g, :], AF.Exp, scale=-0.5)

            # ---------- Phase D: RoPE + per-position rms scale ----------
            with (
                tc.tile_pool(name="ropeps", bufs=2, space="PSUM") as ropeps,
                tc.tile_pool(name="ropetmp", bufs=2) as ropetmp,
            ):
                for m in range(4):
                    g = m % 2
                    patt = 0 if m < 2 else 1
                    for ch in range(2):
                        c0 = ch * 1024
                        qs_ps = ropeps.tile([128, 1024], fp32, tag="qs")
                        for seg in range(2):
                            nc.tensor.matmul(
                                qs_ps[:, seg * 512 : (seg + 1) * 512],
                                perm_sb[:],
                                qkT_sb[:, m, c0 + seg * 512 : c0 + (seg + 1) * 512],
                                start=True,
                                stop=True,
                            )
                        t1 = ropetmp.tile([128, 1024], bf16, tag="t1")
                        nc.vector.tensor_mul(
                            out=t1[:],
                            in0=qkT_sb[:, m, c0 : c0 + 1024],
                            in1=cos_sb[:, c0 : c0 + 1024],
                        )
                        t2 = ropetmp.tile([128, 1024], bf16, tag="t2")
                        nc.vector.tensor_mul(
                            out=t2[:], in0=qs_ps[:], in1=sin_sb[:, c0 : c0 + 1024]
                        )
                        nc.vector.tensor_add(
                            out=qkT_sb[:, m, c0 : c0 + 1024], in0=t1[:], in1=t2[:]
                        )
                        # per-position rsqrt scale via selection matmul broadcast
                        cq_ps = ropeps.tile([128, 1024], fp32, tag="cqb")
                        for seg in range(2):
                            nc.tensor.matmul(
                                cq_ps[:, seg * 512 : (seg + 1) * 512],
                                sel_sb[:, patt, :],
                                cq_sb[:, g, c0 + seg * 512 : c0 + (seg + 1) * 512],
                                start=True,
                                stop=True,
                            )
                        nc.vector.tensor_mul(
                            out=qkT_sb[:, m, c0 : c0 + 1024],
                            in0=qkT_sb[:, m, c0 : c0 + 1024],
                            in1=cq_ps[:],
                        )

            if "noattnv" in ablate:
                nc.vector.memset(oT_sb[:], 0.001)

            # ---------- Phase E: attention (ACT-bound inner loop, pipelined) ----
            with (
                tc.tile_pool(name="scps", bufs=1, space="PSUM") as scps,
                tc.tile_pool(name="otps", bufs=1, space="PSUM") as otps,
                tc.tile_pool(name="expool", bufs=2) as expool,
                tc.tile_pool(name="rbpool", bufs=2) as rbpool,
            ):
                JB = 4  # j-chunks per exp batch
                for p in range(2):
                    for qc in range(2):
                        oT = [
                            otps.tile([65, 1024], fp32, tag=f"ot{h}", name=f"ot{h}")
                            for h in range(2)
                        ]

                        def attnv(b, stage):
                            if "noattnv" in ablate:
                                return
                            for jj in range(JB):
                                j = b * JB + jj
                                for h in range(2):
                                    for s2 in range(2):
                                        nc.tensor.matmul(
                                            oT[h][:, s2 * 512 : (s2 + 1) * 512],
                                            v_sb[:, j, 2 * p + h, 0:65],
                                            stage[:, jj, h, s2 * 512 : (s2 + 1) * 512],
                                            start=(j == 0),
                                            stop=(j == 15),
                                        )

                        pend = None
                        for b in range(16 // JB):
                            stage = expool.tile([128, JB, 2, 1024], bf16, tag="st", name="st")
                            for jj in range(JB):
                                j = b * JB + jj
                                sc = [
                                    scps.tile([128, 1024], fp32, tag=f"sc{h}", name=f"sc{h}")
                                    for h in range(2)
                                ]
                                for h in range(2):
                                    pr = h * 64
                                    for s2 in range(2):
                                        nc.tensor.matmul(
                                            sc[h][:, s2 * 512 : (s2 + 1) * 512],
                                            qkT_sb[pr : pr + 64, 2 + p, j * 128 : (j + 1) * 128],
                                            qkT_sb[
                                                pr : pr + 64,
                                                p,
                                                qc * 1024 + s2 * 512 : qc * 1024 + (s2 + 1) * 512,
                                            ],
                                            start=True,
                                            stop=True,
                                        )
                                for h in range(2):
                                    nc.vector.tensor_copy(
                                        out=stage[:, jj, h, :], in_=sc[h][:]
                                    )
                            # one big exp per batch, in place
                            st2 = stage.rearrange("p a b s -> p (a b s)")
                            if "exp128" in ablate:
                                nc.scalar.activation(
                                    st2[:, 0:128], st2[:, 0:128], AF.Exp, scale=0.125
                                )
                            else:
                                nc.scalar.activation(st2[:], st2[:], AF.Exp, scale=0.125)
                            if pend is not None:
                                attnv(*pend)
                            pend = (b, stage)
                        attnv(*pend)

                        # normalize: 1/denom (psum row 64) broadcast via K=1 matmul
                        for h in range(2 if "noattnv" not in ablate else 0):
                            pr = h * 64
                            dn = rbpool.tile([1, 1024], fp32, tag="dn")
                            nc.vector.tensor_copy(out=dn[:], in_=oT[h][64:65, :])
                            nc.vector.reciprocal_approx_fast(
                                out=rd_sb[0:1, h, :], in_=dn[:]
                            )
                            rb_ps = scps.tile([128, 1024], fp32, tag=f"sc{h}")
                            for s2 in range(2):
                                nc.tensor.matmul(
                                    rb_ps[0:64, s2 * 512 : (s2 + 1) * 512],
                                    ones1_sb[0:1, :],
                                    rd_sb[0:1, h, s2 * 512 : (s2 + 1) * 512],
                                    start=True,
                                    stop=True,
                                )
                            rb = rbpool.tile([64, 1024], bf16, tag="rb")
                            nc.vector.tensor_copy(out=rb[:], in_=rb_ps[0:64, :])
                            nc.vector.tensor_mul(
                                out=oT_sb[pr : pr + 64, p, qc * 1024 : (qc + 1) * 1024],
                                in0=oT[h][0:64, :],
                                in1=rb[:],
                            )

            # ---------- Phase F: output projection ----------
            with (
                tc.tile_pool(name="prps", bufs=2, space="PSUM") as prps,
                tc.tile_pool(name="outpool", bufs=3) as outpool,
            ):
                for si in range(1 if "noproj" in ablate else 16):
                    ob = outpool.tile([128, D], fp32, tag="ob")
                    for ncol in range(2):
                        ps = prps.tile([128, 512], fp32, tag="pr")
                        for kc in range(2):
                            nc.tensor.matmul(
                                ps[:],
                                oT_sb[:, kc, si * 128 : (si + 1) * 128],
                                wpr_sb[:, kc, ncol * 512 : (ncol + 1) * 512],
                                start=(kc == 0),
                                stop=(kc == 1),
                            )
                        nc.vector.tensor_copy(
                            out=ob[:, ncol * 512 : (ncol + 1) * 512], in_=ps[:]
                        )
                    nc.sync.dma_start(out[si * 128 : (si + 1) * 128, :], ob[:])

            if debug:
                nc.sync.dma_start(dbg_qkT[:], qkT_sb[:].rearrange("p a s -> p (a s)"))
                nc.sync.dma_start(dbg_cq[:], cq_sb[:].rearrange("p a s -> p (a s)"))
                nc.sync.dma_start(dbg_v[:], v_sb[:].rearrange("p a b c -> p (a b c)"))
                nc.sync.dma_start(dbg_oT[:], oT_sb[:].rearrange("p a s -> p (a s)"))
                nc.sync.dma_start(dbg_ln[:], ln_sb[:].rearrange("p a s -> p (a s)"))

            rep_stack.close()

    nc.finalize()
    return nc


def _host_inputs(x, Wqkv, bqkv, qg, kg, Wproj, cos, sin):
    """Build the 8 per-core input maps (numpy, host-side sharding/layout)."""
    x = np.asarray(x, dtype=np.float32)
    Wqkv = np.asarray(Wqkv, dtype=np.float32)
    bqkv = np.asarray(bqkv, dtype=np.float32)
    qg = np.asarray(qg, dtype=np.float32)
    kg = np.asarray(kg, dtype=np.float32)
    Wproj = np.asarray(Wproj, dtype=np.float32)
    cos = np.asarray(cos, dtype=np.float32)
    sin = np.asarray(sin, dtype=np.float32)

    cosT2 = np.concatenate([cos.T, cos.T], axis=0).astype(BF16)  # [128, S]
    sf = np.concatenate([-sin[:, : HD // 2], sin[:, HD // 2 :]], axis=1)
    sinT2 = np.concatenate([sf.T, sf.T], axis=0).astype(BF16)  # [128, S]

    permm = np.zeros((128, 128), dtype=BF16)
    for mcol in range(128):
        rot = (mcol + 32) % 64 + 64 * (mcol // 64)
        permm[rot, mcol] = 1.0

    # The two group-members' mask matmuls ACCUMULATE into one [97,512] psum,
    # so their column sets must be disjoint.  mi=0 owns every column except
    # 64/96 (rows 0:64 summed; col 32 takes rows 64:128 — its half1 slot);
    # mi=1 owns only cols 64 (rows 0:64) and 96 (rows 64:128).  Unused rows
    # duplicate mi=0's half0 sum, keeping Ln/Exp finite.
    mask97 = np.zeros((128, 2, 97), dtype=BF16)
    mask97[0:64, 0, :] = 1.0
    mask97[0:64, 0, 32] = 0.0
    mask97[64:128, 0, 32] = 1.0
    mask97[:, 0, 64] = 0.0
    mask97[:, 0, 96] = 0.0
    mask97[0:64, 1, 64] = 1.0
    mask97[64:128, 1, 96] = 1.0

    # sel97[:, patt, :]: out rows 0:64 <- src row 64*patt, rows 64:128 <- row
    # 64*patt+32
    sel97 = np.zeros((97, 2, 128), dtype=BF16)
    sel97[0, 0, 0:64] = 1.0
    sel97[32, 0, 64:128] = 1.0
    sel97[64, 1, 0:64] = 1.0
    sel97[96, 1, 64:128] = 1.0

    ones1 = np.ones((1, 64), dtype=np.float32)

    qg4 = np.tile(qg, HPC)  # [256]
    kg4 = np.tile(kg, HPC)

    xT_b = [np.ascontiguousarray(x[b].T).astype(BF16) for b in range(B)]

    in_maps = []
    for core in range(N_CORES):
        b = core // 4
        hg = core % 4
        cq0 = hg * CW

        wqk_ = np.empty((D, 2 * CW), dtype=np.float32)
        wqk_[:, 0:CW] = Wqkv[:, cq0 : cq0 + CW] * qg4[None, :]
        wqk_[:, CW:] = Wqkv[:, D + cq0 : D + cq0 + CW] * kg4[None, :]

        wv_ = Wqkv[:, 2 * D + cq0 : 2 * D + cq0 + CW]

        # per-partition bias for the 4 transposed qk chunks: chunk m covers
        # output channels m*128..m*128+127 of [q(256) | k(256)]
        bqk_ = np.zeros((128, 4), dtype=np.float32)
        bqkv_qk = np.concatenate(
            [bqkv[cq0 : cq0 + CW] * qg4, bqkv[D + cq0 : D + cq0 + CW] * kg4]
        )
        for m in range(4):
            bqk_[:, m] = bqkv_qk[m * 128 : (m + 1) * 128]

        bvb_ = np.broadcast_to(
            bqkv[2 * D + cq0 : 2 * D + cq0 + CW].astype(BF16)[None, :], (128, CW)
        ).copy()

        in_maps.append(
            {
                "xT": xT_b[b],
                "wqk": wqk_.astype(BF16),
                "wv": wv_.astype(BF16),
                "wpr": np.ascontiguousarray(Wproj[cq0 : cq0 + CW, :]).astype(BF16),
                "cosT2": cosT2,
                "sinT2": sinT2,
                "perm": permm,
                "mask97": mask97.reshape(128, 2 * 97),
                "sel97": sel97.reshape(97, 2 * 128),
                "ones1": ones1,
                "bqk": bqk_,
                "bvb": bvb_,
            }
        )
    return in_maps


_NC_CACHE = None


def kernel(x, Wqkv, bqkv, qg, kg, Wproj, bproj, cos, sin):
    global LAST_RESULTS, _NC_CACHE
    from concourse.bass_utils import run_bass_kernel_spmd

    if _NC_CACHE is None:
        _NC_CACHE = _build_bass()
    nc = _NC_CACHE

    in_maps = _host_inputs(x, Wqkv, bqkv, qg, kg, Wproj, cos, sin)
    res = run_bass_kernel_spmd(nc, in_maps, core_ids=list(range(N_CORES)))
    LAST_RESULTS = res

    bproj = np.asarray(bproj, dtype=np.float32)
    out = np.zeros((B, S, D), dtype=np.float32)
    for b in range(B):
        acc = np.zeros((S, D), dtype=np.float32)
        for i in range(4):
            acc += res.results[4 * b + i]["out"]
        out[b] = acc + bproj[None, :]
    return out
